# revision 22
# baseline (speedup 1.0000x reference)
"""Trainium2 Bass kernel for masked cross-attention (nn_Attention_21440476741938).

Reference computation (b=2, n=4096, n_txt=128, c=1536, c_ctx=4096, h=24, d=64):
    q = x @ Wq;  k = context @ Wk;  v = context @ Wv        (multi-head, d=64)
    out = softmax(q k^T / sqrt(d) + mask) v;  y = out @ Wo
7
Sharding across 8 NeuronCores: core i -> batch b=i//4, quarter j=i%4.
Core computes k/v projections for its 384 feature columns (6 heads), one
packed 4-core AllGather shares full K^T / V per batch, then each core runs
attention + output projection for its 1024 query tokens (all 24 heads).

Schedule (v2, from trace analysis of the 260-285us baseline):
  * PE issue cadence for N=512 bf16 matmuls is already at the streaming
    roofline (216ns warm / 262ns at the 13/16 GPIO power cap) -- LDWEIGHTS
    fully hidden.  All remaining time was scheduling stalls:
  * AllGather was triggered at ~92us (wkv pool-reuse WAR deps + sync-queue
    head blocking behind the gated wo DMA) and exposed a 34us PE gap that
    also re-throttled HAM cold.  Now: wkv is a 6-buf pool whose DMAs are
    gated only on early Q-pass matmuls, KV chunks interleave at passes 3-6,
    and do_ag() triggers at ~48us -- hidden under the remaining Q passes.
    wo reuses wq's SBUF (pool WAR) and its DMA sits after the AG unpack
    entries so the sync queue stays monotone.
  * GpSimd partition_broadcast is fully evicted from the attention phase
    (v2 showed 3.9us per paired bcast, pacing the whole phase and holding
    HAM at K=4/8 for 86us).  The softmax reciprocals are now broadcast with
    two tiny PE outer-product matmuls per chunk (lhsT = ones column) into a
    single PSUM bank, so the whole attention dataflow is PE-self-paced:
      scores -> exp(ACT) -> den(2 rows of one bank, partitions 0/32) ->
      recip(DVE fp32) -> bf16 convert -> bc outer(PE) -> normalize(DVE).
  * O-proj emits: 9 interleave into the attention stream (full-array MMs
    keep HAM warm), the rest follow; emit PSUM is its own pool so emits
    never chain behind attention WARs.
Output is stored bf16 (halves output traffic; host restores fp32).
"""

import ml_dtypes
import numpy as np

import concourse.bass as bass
import concourse.bacc as bacc
import concourse.mybir as mybir
import concourse.tile as tile
from concourse.tile import add_dep_helper
from concourse.bass_utils import run_bass_kernel_spmd

F32 = mybir.dt.float32
BF16 = mybir.dt.bfloat16

B, NQ, NKV, CIN, CCTX, C = 2, 4096, 128, 1536, 4096, 1536
H, D = 24, 64
SCALE = float(D) ** -0.5
NCORES = 8
QTOK = NQ * B // NCORES          # 1024 query tokens per core
FSH = C // 4                     # 384 feature columns per core in phase 1
NCH = C // 128                   # 12 feature chunks
CCH = CCTX // 128                # 32 context-feature chunks
WKVG = 4                         # wkv chunks per DMA group
MASK_NEG = -60.0                 # exp(-60) ~ 8.8e-27: negligible vs valid terms


def build_nc():
    nc = bacc.Bacc("TRN2", target_bir_lowering=False, debug=False,
                   num_devices=NCORES)

    # all big inputs are host-relaid to partition-major so every DMA line is
    # maximally contiguous per partition
    xq2 = nc.dram_tensor("xq2", [128, 2, NCH, 512], BF16, kind="ExternalInput").ap()
    ctxc = nc.dram_tensor("ctxc", [128, CCH, NKV], BF16, kind="ExternalInput").ap()
    wkv = nc.dram_tensor("wkv", [128, CCH, 2 * FSH], BF16, kind="ExternalInput").ap()
    wq = nc.dram_tensor("wq", [128, NCH, C], BF16, kind="ExternalInput").ap()
    wo_bf = nc.dram_tensor("wo_bf", [128, NCH, C], BF16, kind="ExternalInput").ap()
    biasin = nc.dram_tensor("biasin", [NKV, 1], F32, kind="ExternalInput").ap()
    onesin = nc.dram_tensor("onesin", [128, 64], BF16, kind="ExternalInput").ap()
    eyein = nc.dram_tensor("eyein", [128, 128], BF16, kind="ExternalInput").ap()
    yT = nc.dram_tensor("yT", [C, QTOK], BF16, kind="ExternalOutput").ap()

    with tile.TileContext(nc) as tc:
        _build_graph(nc, tc, xq2, ctxc, wkv, wq, wo_bf, biasin, onesin, eyein, yT)
    nc.compile()
    return nc


def _build_graph(nc, tc, xq2, ctxc, wkv, wq, wo_bf, biasin, onesin, eyein, yT):
    Exp = mybir.ActivationFunctionType.Exp

    with (
        tc.tile_pool(name="dram", bufs=1, space="DRAM") as dram,
        tc.tile_pool(name="persist", bufs=1) as persist,
        tc.tile_pool(name="wqwo", bufs=1) as wqwo,
        tc.tile_pool(name="consts", bufs=1) as consts,
        tc.tile_pool(name="wkvp", bufs=6) as wkvp,
        tc.tile_pool(name="p1sb", bufs=1) as p1sb,
        tc.tile_pool(name="expp", bufs=6) as expp,
        tc.tile_pool(name="bcsb", bufs=2) as bcsb,
        tc.tile_pool(name="ytsb", bufs=3) as ytsb,
    ):
        # ---- constants
        ones_t = consts.tile([128, 64], BF16)
        nc.sync.dma_start(ones_t[:], onesin)
        ones_sb = ones_t[:, 0:1]       # (128,1) lhsT for column sums
        bias_sb = consts.tile([NKV, 1], F32)
        nc.sync.dma_start(bias_sb[:], biasin)
        eye_sb = consts.tile([128, 128], BF16)
        nc.sync.dma_start(eye_sb[:], eyein)

        # ---- persistent SBUF tensors
        wq_sb = wqwo.tile([128, NCH * C], BF16, name="wqwo_t")
        xq_sb = persist.tile([128, 2 * NCH * 512], BF16)
        qT_sb = persist.tile([128, NCH * QTOK], BF16)
        outT_sb = persist.tile([128, NCH * QTOK], BF16)
        kT_sb = persist.tile([128, C], BF16)
        v_sb = persist.tile([128, C], BF16)
        ctx_sb = persist.tile([128, CCH * NKV], BF16)

        # ---- DRAM bounce buffers for the packed AllGather
        kv_ag_in = dram.tile([2 * FSH, NKV], BF16)
        kv_full = dram.tile([8 * FSH, NKV], BF16)

        # ---- input DMAs, in consumption order.  xq0(first c-half) + wq g0
        # get the full bus immediately; later wq groups and the xq0 tail are
        # laddered on dribble-matmul progress so each transfer streams at
        # full bandwidth right before the PE needs it (DMA rings run
        # concurrently, so ungated transfers would share the bus and all
        # land late together).  ctx/wkv follow; wo is pushed late (after the
        # AG unpack DMAs) and WAR-gated on wq's last reader.
        nc.sync.dma_start(
            xq_sb[:, 0:3072].rearrange("p (c f) -> p c f", c=NCH // 2),
            xq2[:, 0, 0:NCH // 2])
        d_wq = []
        for g in range(6):
            d_wq.append(nc.sync.dma_start(
                wq_sb[:, 2 * C * g:2 * C * (g + 1)]
                .rearrange("p (c f) -> p c f", c=2),
                wq[:, 2 * g:2 * (g + 1), :]))
            if g == 3:
                d_xq0b = nc.sync.dma_start(
                    xq_sb[:, 3072:6144].rearrange("p (c f) -> p c f",
                                                  c=NCH // 2),
                    xq2[:, 0, NCH // 2:NCH])
        # chain the prologue transfers DMA-to-DMA: each starts when the bus
        # frees from the previous, so they land sequentially at full
        # bandwidth (g0 at ~5us, then one group every ~2.2us) and the
        # c-outer dribble below consumes them as they arrive
        chain = [d_wq[0], d_wq[1], d_wq[2], d_wq[3], d_xq0b,
                 d_wq[4], d_wq[5]]
        for prev, nxt in zip(chain, chain[1:]):
            add_dep_helper(nxt.ins, prev.ins, reason="dma chain: wq/xq0")
        d_ctx = nc.sync.dma_start(
            ctx_sb.rearrange("p (c k) -> p c k", c=CCH), ctxc)
        wkv_tiles = []
        d_wkv = []
        for g in range(CCH // WKVG):
            wkv_t = wkvp.tile([128, WKVG * 2 * FSH], BF16, name="wkv_t")
            d = nc.sync.dma_start(
                wkv_t.rearrange("p (j f) -> p j f", j=WKVG),
                wkv[:, WKVG * g:WKVG * (g + 1), :])
            wkv_tiles.append(wkv_t)
            d_wkv.append(d)
        d_xq1 = nc.sync.dma_start(
            xq_sb[:, 6144:12288].rearrange("p (c f) -> p c f", c=NCH),
            xq2[:, 1])

        # ---- ACT exp table warmup during the DMA-bound prologue; fp32 ones
        # rows (partitions 0/32) used as lhsT of the f32r recip-broadcast
        # outer products; persistent recip staging rows.
        warm_act = consts.tile([128, 1], F32, name="warm_act")
        nc.scalar.activation(warm_act[:], bias_sb[:], Exp,
                             bias=bias_sb[:], scale=0.0)
        rec_f = persist.tile([64, 512], F32, name="rec_f")
        rec_bf = persist.tile([33, 512], BF16, name="rec_bf")

        # ================= projection phase =================
        with (
            tc.tile_pool(name="qtps", bufs=4, space="PSUM") as qtps,
            tc.tile_pool(name="kvps", bufs=1, space="PSUM") as kvps,
            tc.tile_pool(name="trps", bufs=2, space="PSUM") as trps,
        ):
            def q_pass(qs, fc):
                q_ps = qtps.tile([128, 512], F32, name="q_ps")
                first = last = None
                for c in range(NCH):
                    last = nc.tensor.matmul(
                        q_ps[:],
                        wq_sb[:, C * c + 128 * fc:C * c + 128 * (fc + 1)],
                        xq_sb[:, 6144 * qs + 512 * c:6144 * qs + 512 * (c + 1)],
                        start=(c == 0), stop=(c == NCH - 1))
                    if first is None:
                        first = last
                nc.scalar.copy(
                    qT_sb[:, QTOK * fc + 512 * qs:QTOK * fc + 512 * (qs + 1)],
                    q_ps[:])
                return first, last

            # ---- Q projection first query half with K/V chunks interleaved
            # early (passes 3-6) so the AllGather launches ~48us and flies
            # under the remaining Q passes.
            k_ps = kvps.tile([NKV, FSH], F32)
            v_ps = kvps.tile([NKV, FSH], F32)

            def kv_chunk(c):
                wkv_t = wkv_tiles[c // WKVG]
                j = c % WKVG
                nc.tensor.matmul(k_ps[:], ctx_sb[:, NKV * c:NKV * (c + 1)],
                                 wkv_t[:, 2 * FSH * j:2 * FSH * j + FSH],
                                 start=(c == 0), stop=(c == CCH - 1))
                nc.tensor.matmul(v_ps[:], ctx_sb[:, NKV * c:NKV * (c + 1)],
                                 wkv_t[:, 2 * FSH * j + FSH:2 * FSH * (j + 1)],
                                 start=(c == 0), stop=(c == CCH - 1))

            def do_ag():
                # v shard: natural layout -> second half of the packed AG
                # input.  Staging copies go on DVE (idle here; ACT is busy
                # with qT copies) so the collective triggers ASAP.
                v_stage = p1sb.tile([NKV, FSH], BF16)
                nc.vector.tensor_copy(v_stage[:], v_ps[:])
                v_dst = (kv_ag_in.rearrange("(x pk) k -> x (pk k)", x=2)[1:2, :]
                         .rearrange("o (p f) -> (o p) f", p=128))
                nc.sync.dma_start(v_dst, v_stage[:])

                # k shard: transpose (128kv, 384f) -> (384f, 128kv)
                k_nat = p1sb.tile([NKV, FSH], BF16)
                nc.vector.tensor_copy(k_nat[:], k_ps[:])
                kT_stage = p1sb.tile([128, 3 * NKV], BF16)
                for s in range(3):
                    kt_ps = trps.tile([128, 128], BF16, name="kt_ps")
                    nc.tensor.transpose(kt_ps[:],
                                        k_nat[:, 128 * s:128 * (s + 1)],
                                        eye_sb[:])
                    nc.vector.tensor_copy(kT_stage[:, 128 * s:128 * (s + 1)],
                                          kt_ps[:])
                nc.sync.dma_start(
                    kv_ag_in[0:FSH, :].rearrange("(s p) k -> p s k", p=128),
                    kT_stage.rearrange("p (s k) -> p s k", s=3))

                groups = [[0, 1, 2, 3], [4, 5, 6, 7]]
                nc.gpsimd.collective_compute(
                    "AllGather", mybir.AluOpType.bypass,
                    replica_groups=groups,
                    ins=[kv_ag_in[:].opt()], outs=[kv_full[:].opt()])

                # unpack: rank g's kT rows -> kT_sb blocks 3g..3g+2 (one
                # DMA per rank keeps the post-collective path short)
                for g in range(4):
                    nc.sync.dma_start(
                        kT_sb[:, FSH * g:FSH * (g + 1)]
                        .rearrange("p (s k) -> p s k", s=3),
                        kv_full[768 * g:768 * g + FSH, :]
                        .rearrange("(s p) k -> p s k", p=128))
                    v_src = (kv_full
                             .rearrange("(gg x pk) k -> gg x (pk k)",
                                        gg=4, x=2)
                             [g:g + 1, 1:2, :]
                             .rearrange("go o (p f) -> (go o p) f", p=128))
                    nc.sync.dma_start(v_sb[:, FSH * g:FSH * (g + 1)], v_src)

            # ---- prologue dribble: fc 0-3 of qs=0 run c-outer across all 4
            # qtps banks, so the PE consumes each wq DMA group as it lands
            # instead of idling until the whole weight is resident.
            q_ps_d = [qtps.tile([128, 512], F32, name="q_ps")
                      for _ in range(4)]
            dr_mms = []
            for c in range(NCH):
                for fc in range(4):
                    dr_mms.append(nc.tensor.matmul(
                        q_ps_d[fc][:],
                        wq_sb[:, C * c + 128 * fc:C * c + 128 * (fc + 1)],
                        xq_sb[:, 512 * c:512 * (c + 1)],
                        start=(c == 0), stop=(c == NCH - 1)))
            dr_last = dr_mms[-1]
            for fc in range(4):
                nc.scalar.copy(qT_sb[:, QTOK * fc:QTOK * fc + 512],
                               q_ps_d[fc][:])



            # KV chunks run after passes 5-8; the AllGather chain is emitted
            # right after the last chunk so the collective (11.5us trigger
            # latency + 9-31us run) lands before the attention phase needs it.
            kv_after = {5: range(0, 8), 6: range(8, 16),
                        7: range(16, 24), 8: range(24, 32)}
            pass_mms = {}
            for fc in range(4, NCH):
                _, l = q_pass(0, fc)
                pass_mms[fc] = l
                for c in kv_after.get(fc, ()):
                    kv_chunk(c)
                if fc == 8:
                    do_ag()

            # DMA gating: ctx + first wkv groups go right as the dribble ends
            # (bus is free once wq/xq0 landed); later wkv groups and xq1
            # ladder behind the first full passes.
            add_dep_helper(d_ctx.ins, dr_last.ins, reason="dma order: ctx")
            for g in range(len(d_wkv)):
                gate = dr_last if g < 2 else pass_mms[4 + (g - 2) // 2]
                add_dep_helper(d_wkv[g].ins, gate.ins, reason="dma order: wkv")
            add_dep_helper(d_xq1.ins, pass_mms[4].ins, reason="dma order: xq1")

            # ---- Q projection second query half: covers the AllGather
            for fc in range(NCH):
                q_pass(1, fc)

        # wo lands in wq's SBUF slot (pool WAR on the last Q-proj reader);
        # pushed after the AG unpack DMAs so the sync queue stays monotone.
        wo_sb = wqwo.tile([128, NCH * C], BF16, name="wqwo_t")
        nc.sync.dma_start(wo_sb.rearrange("p (c f) -> p c f", c=NCH), wo_bf)

        # ================= attention + output projection =================
        with (
            tc.tile_pool(name="scps", bufs=2, space="PSUM") as scps,
            tc.tile_pool(name="denp", bufs=1, space="PSUM") as denp,
            tc.tile_pool(name="ovp", bufs=2, space="PSUM") as ovp,
            tc.tile_pool(name="bcp", bufs=1, space="PSUM") as bcp,
            tc.tile_pool(name="ytp", bufs=2, space="PSUM") as ytp,
        ):
            # 24 (qt, c2) chunks, 2-stage pipeline, all PE-self-paced:
            #   A(i):    scores pair (row-tiled, concurrent) + exp (ACT)
            #   BC(i-1): den rows (one bank, partitions 0/32) -> recip (DVE)
            #            -> bf16 -> attn.v + recip outer-broadcast (PE)
            #            -> normalize muls (DVE)
            chunks = [(qt, c2) for qt in range(2) for c2 in range(NCH)]
            n = len(chunks)
            state = {}

            def stage_a(i):
                qt, c2 = chunks[i]
                exps = []
                for hh in range(2):
                    sc_ps = scps.tile([NKV, 512], F32, name="sc_ps")
                    nc.tensor.matmul(
                        sc_ps[:],
                        kT_sb[64 * hh:64 * hh + 64, 128 * c2:128 * (c2 + 1)],
                        qT_sb[64 * hh:64 * hh + 64,
                              QTOK * c2 + 512 * qt:QTOK * c2 + 512 * qt + 512],
                        start=True, stop=True)
                    exp_sb = expp.tile([NKV, 512], BF16, name="exp_sb")
                    nc.scalar.activation(exp_sb[:], sc_ps[:], Exp,
                                         bias=bias_sb[:], scale=SCALE)
                    exps.append(exp_sb)
                state[i] = {"exps": exps}

            def stage_bc(i):
                qt, c2 = chunks[i]
                exps = state.pop(i)["exps"]
                den2 = denp.tile([64, 512], F32, name="den2")
                nc.tensor.matmul(den2[0:1, :], ones_sb, exps[0][:],
                                 start=True, stop=True)
                nc.tensor.matmul(den2[32:33, :], ones_sb, exps[1][:],
                                 start=True, stop=True)
                # one dense reciprocal covers both den rows (partitions 0 and
                # 32; rows 1-31 are never-read junk, cost is free-size bound)
                nc.vector.reciprocal_approx_fast(rec_f[0:33, :],
                                                 den2[0:33, :])
                # bf16 convert on ACT: balances engine loads (DVE carries
                # recip + bc copy + mul; ACT carries exp pair + this)
                nc.scalar.copy(rec_bf[:], rec_f[0:33, :])
                ov_ps = ovp.tile([128, 512], F32, name="ov_ps")
                for hh in range(2):
                    h = 2 * c2 + hh
                    nc.tensor.matmul(
                        ov_ps[64 * hh:64 * hh + 64, :],
                        v_sb[:, 64 * h:64 * h + 64],
                        exps[hh][:], start=True, stop=True)
                # broadcast 1/den across partitions: bf16 outer products
                # (rows 0-63 <- head0 recip, rows 64-127 <- head1 recip)
                bc_ps = bcp.tile([128, 512], F32, name="bc_ps")
                nc.tensor.matmul(bc_ps[0:64, :], ones_t[0:1, 0:64],
                                 rec_bf[0:1, :], start=True, stop=True)
                nc.tensor.matmul(bc_ps[64:128, :], ones_t[32:33, 0:64],
                                 rec_bf[32:33, :], start=True, stop=True)
                bc_sb = bcsb.tile([128, 512], F32, name="bc_sb")
                nc.vector.tensor_copy(bc_sb[:], bc_ps[:])
                ocol = QTOK * c2 + 512 * qt
                nc.vector.tensor_mul(outT_sb[:, ocol:ocol + 512],
                                     ov_ps[:], bc_sb[:])

            def emit_yt(oc, qt):
                y_ps = ytp.tile([128, 512], F32, name="y_ps")
                for c in range(NCH):
                    nc.tensor.matmul(
                        y_ps[:],
                        wo_sb[:, C * c + 128 * oc:C * c + 128 * (oc + 1)],
                        outT_sb[:, QTOK * c + 512 * qt:QTOK * c + 512 * qt + 512],
                        start=(c == 0), stop=(c == NCH - 1))
                y_sb = ytsb.tile([128, 512], BF16, name="y_sb")
                nc.scalar.copy(y_sb[:], y_ps[:])
                nc.sync.dma_start(
                    yT[128 * oc:128 * (oc + 1), 512 * qt:512 * qt + 512],
                    y_sb[:])

            # chunks + interleaved qt=0 emits from the earliest legal point
            # (emit oc needs all 12 qt=0 chunks normalized, i.e. iteration
            # 12); the full-array emit MMs keep the HAM activity monitor
            # warm through the second half of the attention phase.
            for i in range(n + 1):
                if i < n:
                    stage_a(i)
                if i >= 1:
                    stage_bc(i - 1)
                if 12 <= i <= 23:
                    emit_yt(i - 12, 0)

            for oc in range(NCH):
                emit_yt(oc, 1)


_NC_CACHE = None


def _get_nc():
    global _NC_CACHE
    if _NC_CACHE is None:
        _NC_CACHE = build_nc()
    return _NC_CACHE


def make_in_maps(x, context, context_mask, Wq, Wk, Wv, Wo):
    x = np.ascontiguousarray(np.asarray(x, dtype=np.float32))
    context = np.asarray(context, dtype=np.float32)
    context_mask = np.asarray(context_mask)
    Wq = np.ascontiguousarray(np.asarray(Wq, dtype=np.float32))
    Wk = np.asarray(Wk, dtype=np.float32)
    Wv = np.asarray(Wv, dtype=np.float32)
    Wo = np.ascontiguousarray(np.asarray(Wo, dtype=np.float32))

    bf = ml_dtypes.bfloat16
    eye = np.eye(128, dtype=bf)
    ones = np.ones((128, 64), dtype=bf)
    # partition-major layouts: arr[p, ...] is contiguous per partition
    wq_bf = np.ascontiguousarray(
        Wq.reshape(NCH, 128, C).transpose(1, 0, 2).astype(bf))
    wo_bf = np.ascontiguousarray(
        Wo.reshape(NCH, 128, C).transpose(1, 0, 2).astype(bf))
    ctx_by_b = [np.ascontiguousarray(context[b].T.reshape(CCH, 128, NKV)
                                     .transpose(1, 0, 2).astype(bf))
                for b in range(B)]
    in_maps = []
    for i in range(NCORES):
        b, j = i // 4, i % 4
        bias = np.where(context_mask[b], 0.0, MASK_NEG).astype(np.float32)[:, None]
        xTc = x[b, QTOK * j:QTOK * (j + 1), :].T          # (1536, 1024)
        xq2 = np.ascontiguousarray(
            xTc.reshape(NCH, 128, 2, 512).transpose(1, 2, 0, 3).astype(bf))
        wkv = np.ascontiguousarray(
            np.concatenate([Wk[:, FSH * j:FSH * (j + 1)],
                            Wv[:, FSH * j:FSH * (j + 1)]], axis=1)
            .reshape(CCH, 128, 2 * FSH).transpose(1, 0, 2).astype(bf))
        in_maps.append({
            "xq2": xq2,
            "ctxc": ctx_by_b[b],
            "wkv": wkv,
            "wq": wq_bf,
            "wo_bf": wo_bf,
            "biasin": bias,
            "onesin": ones,
            "eyein": eye,
        })
    return in_maps


def kernel(x, context, context_mask, Wq, Wk, Wv, Wo):
    in_maps = make_in_maps(x, context, context_mask, Wq, Wk, Wv, Wo)
    nc = _get_nc()
    res = run_bass_kernel_spmd(nc, in_maps, core_ids=list(range(NCORES)))

    y = np.empty((B, NQ, C), dtype=np.float32)
    for i in range(NCORES):
        b, j = i // 4, i % 4
        y[b, QTOK * j:QTOK * (j + 1), :] = res.results[i]["yT"].T
    return y
